# revision 6
# baseline (speedup 1.0000x reference)
"""MinGRU block kernel for Trainium2 (8 NeuronCores, data-parallel).

Sharding: 8 devices = (batch 4) x (sequence halves 2). Each device processes
2048 "own" tokens plus a 256-token burn-in prefix for the GRU scan (the linear
recurrence h_t = c*h + v with c<1 forgets its initial condition below fp32
resolution well within 256 steps), so there is no cross-device communication.
Device layout is [D, T] (channels on partitions); matmuls run in float32r.
"""
import sys
sys.path.insert(0, '/opt/trn_rl_repo')

import math
import numpy as np
from contextlib import ExitStack

import concourse.bacc as bacc
import concourse.tile as tile
import concourse.mybir as mybir

AF = mybir.ActivationFunctionType
ALU = mybir.AluOpType
F32 = mybir.dt.float32
F32R = mybir.dt.float32r

D = 1024
EFF = 4096
T_OWN = 2048
BURN = 256
TT = 256          # token tile
K = 3             # dwconv taps
B = 4
L = 4096
N_CORES = 8
RSQRT_EPS = 1e-30


def build_program(D=D, EFF=EFF, T_OWN=T_OWN, BURN=BURN, TT=TT):
    DC = D // 128            # d chunks
    EGC = 2 * D // 128       # gru output chunks
    EFC = EFF // 128         # ff hidden chunks
    T_SCAN = BURN + T_OWN
    T_X = T_SCAN + 2
    NT = T_SCAN // TT
    NTO = T_OWN // TT
    OWN_T0 = BURN // TT      # first tile with own tokens
    assert T_SCAN % TT == 0 and T_OWN % TT == 0 and BURN % TT == 0

    nc = bacc.Bacc("TRN2", target_bir_lowering=False, debug=False)

    # ---- DRAM I/O ----
    x_in = nc.dram_tensor("x_in", [D, T_X], F32, kind="ExternalInput").ap()
    w_pw_t = nc.dram_tensor("w_pw_t", [D, D], F32R, kind="ExternalInput").ap()
    w_gru_t = nc.dram_tensor("w_gru_t", [D, 2 * D], F32R, kind="ExternalInput").ap()
    w_ff1_t = nc.dram_tensor("w_ff1_t", [D, EFF], F32R, kind="ExternalInput").ap()
    w_ff2_t = nc.dram_tensor("w_ff2_t", [EFF, D], F32R, kind="ExternalInput").ap()
    w_dw = nc.dram_tensor("w_dw", [128, DC * K], F32, kind="ExternalInput").ap()
    b_eff = nc.dram_tensor("b_eff", [128, DC], F32, kind="ExternalInput").ap()
    b1_in = nc.dram_tensor("b1_in", [128, EFC], F32, kind="ExternalInput").ap()
    b2_in = nc.dram_tensor("b2_in", [128, DC], F32, kind="ExternalInput").ap()
    mask_in = nc.dram_tensor("mask_in", [128, 1], F32, kind="ExternalInput").ap()
    ones_in = nc.dram_tensor("ones_in", [128, 1], F32R, kind="ExternalInput").ap()
    half_in = nc.dram_tensor("half_in", [128, 1], F32, kind="ExternalInput").ap()

    out_x = nc.dram_tensor("out_x", [D, T_OWN], F32, kind="ExternalOutput").ap()
    out_h = nc.dram_tensor("out_h", [D, 1], F32, kind="ExternalOutput").ap()

    # ---- DRAM scratch ----
    x1_d = nc.dram_tensor("x1_d", [D, T_SCAN], F32).ap()
    x2_d = nc.dram_tensor("x2_d", [D, T_OWN], F32).ap()
    xn3_d = nc.dram_tensor("xn3_d", [D, T_OWN], F32R).ap()
    r2_d = nc.dram_tensor("r2_d", [1, T_SCAN], F32).ap()

    with tile.TileContext(nc) as tc, ExitStack() as top:
        const_pool = top.enter_context(tc.tile_pool(name="consts", bufs=1))
        ones_r = const_pool.tile([128, 1], F32R)
        nc.sync.dma_start(ones_r[:], ones_in[:])
        mask_sb = const_pool.tile([128, 1], F32)
        nc.sync.dma_start(mask_sb[:], mask_in[:])
        half_sb = const_pool.tile([128, 1], F32)
        nc.sync.dma_start(half_sb[:], half_in[:])

        # ============ Phase A: conv branch -> x1 = pwconv(dwconv(rmsnorm(x))) + x
        with ExitStack() as ctx:
            wpool = ctx.enter_context(tc.tile_pool(name="A_w", bufs=1))
            xpool = ctx.enter_context(tc.tile_pool(name="A_x", bufs=1))
            work = ctx.enter_context(tc.tile_pool(name="A_work", bufs=2))
            ps_s = ctx.enter_context(tc.tile_pool(name="A_ps_s", bufs=2, space="PSUM"))
            ps_pw = ctx.enter_context(tc.tile_pool(name="A_ps_pw", bufs=4, space="PSUM"))

            wpw = []
            wdw = []
            beff = []
            for c in range(DC):
                wt = wpool.tile([128, D], F32R, tag=f"wpw{c}")
                nc.sync.dma_start(wt[:], w_pw_t[c * 128:(c + 1) * 128, :])
                wpw.append(wt)
            wdw_t = wpool.tile([128, DC * K], F32)
            nc.sync.dma_start(wdw_t[:], w_dw[:])
            beff_t = wpool.tile([128, DC], F32)
            nc.sync.dma_start(beff_t[:], b_eff[:])

            xs = []
            for c in range(DC):
                xt = xpool.tile([128, T_X], F32, tag=f"x{c}")
                nc.sync.dma_start(xt[:], x_in[c * 128:(c + 1) * 128, :])
                xs.append(xt)

            W = TT + 2  # window with conv halo
            for i in range(NT):
                xo = TT * i          # x-local window start
                so = TT * i          # scan-local output start
                # rms denom: s = sum_d x^2 over window
                s_ps = ps_s.tile([1, W], F32, tag="s")
                sqs = []
                for c in range(DC):
                    sq = work.tile([128, W], F32R, tag=f"sq{c}")
                    nc.vector.tensor_tensor(sq[:], xs[c][:, xo:xo + W],
                                            xs[c][:, xo:xo + W], ALU.mult)
                    sqs.append(sq)
                for c in range(DC):
                    nc.tensor.matmul(s_ps[:], ones_r[:], sqs[c][:],
                                     start=(c == 0), stop=(c == DC - 1))
                s_e = work.tile([1, W], F32, tag="s_e")
                nc.vector.tensor_scalar_add(s_e[:], s_ps[:], RSQRT_EPS * D)
                r1l = work.tile([1, W], F32, tag="r1l")
                nc.scalar.activation(r1l[:], s_e[:], AF.Ln, scale=1.0 / D)
                r1 = work.tile([1, W], F32, tag="r1")
                nc.scalar.activation(r1[:], r1l[:], AF.Exp, scale=-0.5)
                rb = work.tile([128, W], F32, tag="rb")
                nc.gpsimd.partition_broadcast(rb[:], r1[:])

                xn1 = []
                for c in range(DC):
                    xn = work.tile([128, W], F32R, tag=f"xn1_{c}")
                    nc.vector.tensor_tensor(xn[:], xs[c][:, xo:xo + W], rb[:],
                                            ALU.mult)
                    xn1.append(xn)

                ys = []
                for c in range(DC):
                    y0 = work.tile([128, TT], F32, tag=f"y0_{c}")
                    nc.vector.tensor_scalar_mul(
                        y0[:], xn1[c][:, 0:TT].bitcast(F32),
                        wdw_t[:, c * K:c * K + 1])
                    y1 = work.tile([128, TT], F32, tag=f"y1_{c}")
                    nc.vector.scalar_tensor_tensor(
                        y1[:], xn1[c][:, 1:TT + 1].bitcast(F32),
                        wdw_t[:, c * K + 1:c * K + 2], y0[:], ALU.mult, ALU.add)
                    y2 = work.tile([128, TT], F32R, tag=f"y2_{c}")
                    nc.vector.scalar_tensor_tensor(
                        y2[:], xn1[c][:, 2:TT + 2].bitcast(F32),
                        wdw_t[:, c * K + 2:c * K + 3], y1[:], ALU.mult, ALU.add)
                    ys.append(y2)

                for m in range(DC):
                    pw_ps = ps_pw.tile([128, TT], F32, tag="pw")
                    for c in range(DC):
                        nc.tensor.matmul(pw_ps[:],
                                         wpw[c][:, m * 128:(m + 1) * 128],
                                         ys[c][:], start=(c == 0),
                                         stop=(c == DC - 1))
                    x1t = work.tile([128, TT], F32, tag="x1t")
                    nc.vector.scalar_tensor_tensor(
                        x1t[:], pw_ps[:], beff_t[:, m:m + 1],
                        xs[m][:, xo + 2:xo + 2 + TT], ALU.add, ALU.add)
                    nc.sync.dma_start(x1_d[m * 128:(m + 1) * 128, so:so + TT],
                                      x1t[:])

        # ============ Phase B0: r2 row for rmsnorm(x1)
        with ExitStack() as ctx:
            work = ctx.enter_context(tc.tile_pool(name="B0_work", bufs=3))
            ps_s = ctx.enter_context(tc.tile_pool(name="B0_ps", bufs=2, space="PSUM"))
            for i in range(NT):
                so = TT * i
                s_ps = ps_s.tile([1, TT], F32, tag="s")
                for c in range(DC):
                    x1t = work.tile([128, TT], F32, tag="x1t")
                    nc.sync.dma_start(x1t[:],
                                      x1_d[c * 128:(c + 1) * 128, so:so + TT])
                    sq = work.tile([128, TT], F32R, tag="sq")
                    nc.vector.tensor_tensor(sq[:], x1t[:], x1t[:], ALU.mult)
                    nc.tensor.matmul(s_ps[:], ones_r[:], sq[:],
                                     start=(c == 0), stop=(c == DC - 1))
                s_e = work.tile([1, TT], F32, tag="s_e")
                nc.vector.tensor_scalar_add(s_e[:], s_ps[:], RSQRT_EPS * D)
                r2l = work.tile([1, TT], F32, tag="r2l")
                nc.scalar.activation(r2l[:], s_e[:], AF.Ln, scale=1.0 / D)
                r2 = work.tile([1, TT], F32, tag="r2")
                nc.scalar.activation(r2[:], r2l[:], AF.Exp, scale=-0.5)
                nc.sync.dma_start(r2_d[:, so:so + TT], r2[:])

        # ============ Phase B: GRU branch -> x2 = minGRU(rmsnorm(x1)) + x1
        with ExitStack() as ctx:
            wpool = ctx.enter_context(tc.tile_pool(name="B_w", bufs=1))
            w1 = ctx.enter_context(tc.tile_pool(name="B_t1", bufs=1))
            w2 = ctx.enter_context(tc.tile_pool(name="B_t2", bufs=2))
            ps_hg = ctx.enter_context(tc.tile_pool(name="B_ps", bufs=4, space="PSUM"))

            wgru = []
            for c in range(DC):
                wt = wpool.tile([128, 2 * D], F32R, tag=f"wgru{c}")
                nc.sync.dma_start(wt[:], w_gru_t[c * 128:(c + 1) * 128, :])
                wgru.append(wt)

            h_prev = [None] * DC
            for i in range(NT):
                so = TT * i
                x1ts = []
                for c in range(DC):
                    x1t = w2.tile([128, TT], F32, tag=f"x1t{c}")
                    nc.sync.dma_start(x1t[:],
                                      x1_d[c * 128:(c + 1) * 128, so:so + TT])
                    x1ts.append(x1t)
                r2row = w1.tile([1, TT], F32, tag="r2row")
                nc.sync.dma_start(r2row[:], r2_d[:, so:so + TT])
                rb2 = w1.tile([128, TT], F32, tag="rb2")
                nc.gpsimd.partition_broadcast(rb2[:], r2row[:])
                xn2 = []
                for c in range(DC):
                    xn = w1.tile([128, TT], F32R, tag=f"xn2_{c}")
                    nc.vector.tensor_tensor(xn[:], x1ts[c][:], rb2[:], ALU.mult)
                    xn2.append(xn)

                gs = []
                for m in range(DC):           # hidden chunks
                    hg_ps = ps_hg.tile([128, TT], F32, tag="hg")
                    for c in range(DC):
                        nc.tensor.matmul(hg_ps[:],
                                         wgru[c][:, m * 128:(m + 1) * 128],
                                         xn2[c][:], start=(c == 0),
                                         stop=(c == DC - 1))
                    sg = w1.tile([128, TT], F32, tag=f"sg{m}")
                    nc.scalar.activation(sg[:], hg_ps[:], AF.Sigmoid)
                    gt = w1.tile([128, TT], F32, tag=f"gt{m}")
                    nc.scalar.activation(gt[:], hg_ps[:], AF.Identity,
                                         bias=half_sb[:])
                    g = w1.tile([128, TT], F32, tag=f"g{m}")
                    nc.vector.tensor_tensor(g[:], gt[:], sg[:], ALU.max)
                    gs.append(g)

                hs = []
                for mc in range(DC):          # gate chunks
                    m = DC + mc
                    hg_ps = ps_hg.tile([128, TT], F32, tag="hg")
                    for c in range(DC):
                        nc.tensor.matmul(hg_ps[:],
                                         wgru[c][:, m * 128:(m + 1) * 128],
                                         xn2[c][:], start=(c == 0),
                                         stop=(c == DC - 1))
                    ct = w1.tile([128, TT], F32, tag=f"ct{mc}")
                    nc.scalar.activation(ct[:], hg_ps[:], AF.Sigmoid, scale=-1.0)
                    z = w1.tile([128, TT], F32, tag=f"z{mc}")
                    nc.scalar.activation(z[:], hg_ps[:], AF.Sigmoid)
                    v = w1.tile([128, TT], F32, tag=f"v{mc}")
                    nc.vector.tensor_tensor(v[:], z[:], gs[mc][:], ALU.mult)
                    if i == OWN_T0:
                        # reset decay at the first real token (no-op when mask=1)
                        nc.vector.tensor_scalar_mul(ct[:, 0:1], ct[:, 0:1],
                                                    mask_sb[:])
                    h = w2.tile([128, TT], F32, tag=f"h{mc}")
                    init = 0.0 if i == 0 else h_prev[mc][:, TT - 1:TT]
                    nc.vector.tensor_tensor_scan(h[:], ct[:], v[:], init,
                                                 ALU.mult, ALU.add)
                    hs.append(h)
                    if i >= OWN_T0:
                        x2t = w1.tile([128, TT], F32, tag=f"x2t{mc}")
                        nc.vector.tensor_tensor(x2t[:], hs[mc][:], x1ts[mc][:],
                                                ALU.add)
                        nc.sync.dma_start(
                            x2_d[mc * 128:(mc + 1) * 128,
                                 so - BURN:so - BURN + TT], x2t[:])
                    if i == NT - 1:
                        nc.sync.dma_start(out_h[mc * 128:(mc + 1) * 128, :],
                                          hs[mc][:, TT - 1:TT])
                h_prev = hs

        # ============ Phase C0: xn3 = rmsnorm(x2) (f32r, to DRAM)
        with ExitStack() as ctx:
            work = ctx.enter_context(tc.tile_pool(name="C0_work", bufs=3))
            ps_s = ctx.enter_context(tc.tile_pool(name="C0_ps", bufs=2, space="PSUM"))
            for i in range(NTO):
                so = TT * i
                x2ts = []
                sqs = []
                for c in range(DC):
                    x2t = work.tile([128, TT], F32, tag=f"x2t{c}")
                    nc.sync.dma_start(x2t[:],
                                      x2_d[c * 128:(c + 1) * 128, so:so + TT])
                    x2ts.append(x2t)
                s_ps = ps_s.tile([1, TT], F32, tag="s")
                for c in range(DC):
                    sq = work.tile([128, TT], F32R, tag="sq")
                    nc.vector.tensor_tensor(sq[:], x2ts[c][:], x2ts[c][:],
                                            ALU.mult)
                    nc.tensor.matmul(s_ps[:], ones_r[:], sq[:],
                                     start=(c == 0), stop=(c == DC - 1))
                s_e = work.tile([1, TT], F32, tag="s_e")
                nc.vector.tensor_scalar_add(s_e[:], s_ps[:], RSQRT_EPS * D)
                r3l = work.tile([1, TT], F32, tag="r3l")
                nc.scalar.activation(r3l[:], s_e[:], AF.Ln, scale=1.0 / D)
                r3 = work.tile([1, TT], F32, tag="r3")
                nc.scalar.activation(r3[:], r3l[:], AF.Exp, scale=-0.5)
                rb3 = work.tile([128, TT], F32, tag="rb3")
                nc.gpsimd.partition_broadcast(rb3[:], r3[:])
                for c in range(DC):
                    xn = work.tile([128, TT], F32R, tag=f"xn3_{c}")
                    nc.vector.tensor_tensor(xn[:], x2ts[c][:], rb3[:], ALU.mult)
                    nc.sync.dma_start(xn3_d[c * 128:(c + 1) * 128, so:so + TT],
                                      xn[:])

        # ============ Phase C: FF branch fused -> out = W2 gelu(W1 xn3 + b1) + b2 + x2
        # split into 2 E-halves (weights don't all fit in SBUF); half-0 partial
        # (+ b2 + x2 residual) goes to DRAM, half-1 adds it back.
        EH = EFC // 2
        acc_d = nc.dram_tensor("acc_d", [D, T_OWN], F32).ap()
        with ExitStack() as ctx:
            wpool = ctx.enter_context(tc.tile_pool(name="C_w", bufs=1))
            work = ctx.enter_context(tc.tile_pool(name="C_work", bufs=2))
            h1p = ctx.enter_context(tc.tile_pool(name="C_h1", bufs=3))
            ps_f1 = ctx.enter_context(tc.tile_pool(name="C_ps1", bufs=3, space="PSUM"))
            ps_f2 = ctx.enter_context(tc.tile_pool(name="C_ps2", bufs=1, space="PSUM"))

            b1t = wpool.tile([128, EFC], F32)
            nc.sync.dma_start(b1t[:], b1_in[:])
            b2t = wpool.tile([128, DC], F32)
            nc.sync.dma_start(b2t[:], b2_in[:])

            for eh in range(2):
                w1s = []
                for c in range(DC):
                    w1t = wpool.tile([128, EFF // 2], F32R, tag=f"w1_{c}")
                    nc.sync.dma_start(
                        w1t[:], w_ff1_t[c * 128:(c + 1) * 128,
                                        eh * (EFF // 2):(eh + 1) * (EFF // 2)])
                    w1s.append(w1t)
                w2s = []
                for e in range(EH):
                    ge = eh * EH + e
                    w2t = wpool.tile([128, D], F32R, tag=f"w2_{e}")
                    nc.sync.dma_start(w2t[:],
                                      w_ff2_t[ge * 128:(ge + 1) * 128, :])
                    w2s.append(w2t)

                for i in range(NTO):
                    so = TT * i
                    xn3ts = []
                    for c in range(DC):
                        xt = work.tile([128, TT], F32R, tag=f"xn3t{c}")
                        nc.sync.dma_start(
                            xt[:], xn3_d[c * 128:(c + 1) * 128, so:so + TT])
                        xn3ts.append(xt)
                    res_ts = []
                    for c in range(DC):
                        rt = work.tile([128, TT], F32, tag=f"res{c}")
                        src_d = x2_d if eh == 0 else acc_d
                        nc.sync.dma_start(
                            rt[:], src_d[c * 128:(c + 1) * 128, so:so + TT])
                        res_ts.append(rt)

                    f2_ps = []
                    for j in range(DC // 2):
                        f2b = ps_f2.tile([128, 2 * TT], F32, tag=f"f2_{j}")
                        f2_ps.append(f2b)
                    for me in range(EH):
                        f1 = ps_f1.tile([128, TT], F32, tag="f1")
                        for c in range(DC):
                            nc.tensor.matmul(f1[:],
                                             w1s[c][:, me * 128:(me + 1) * 128],
                                             xn3ts[c][:], start=(c == 0),
                                             stop=(c == DC - 1))
                        h1 = h1p.tile([128, TT], F32R, tag="h1")
                        nc.scalar.activation(h1[:], f1[:], AF.Gelu,
                                             bias=b1t[:, eh * EH + me:
                                                      eh * EH + me + 1])
                        for md in range(DC):
                            reg = f2_ps[md // 2][:, (md % 2) * TT:
                                                 (md % 2 + 1) * TT]
                            nc.tensor.matmul(reg,
                                             w2s[me][:, md * 128:(md + 1) * 128],
                                             h1[:],
                                             start=(me == 0 and md % 2 == 0),
                                             stop=(me == EH - 1 and md % 2 == 1))
                    for md in range(DC):
                        reg = f2_ps[md // 2][:, (md % 2) * TT:(md % 2 + 1) * TT]
                        ot = work.tile([128, TT], F32, tag="ot")
                        if eh == 0:
                            # partial + b2 + x2 residual -> acc_d
                            nc.vector.scalar_tensor_tensor(
                                ot[:], reg, b2t[:, md:md + 1], res_ts[md][:],
                                ALU.add, ALU.add)
                            nc.sync.dma_start(
                                acc_d[md * 128:(md + 1) * 128, so:so + TT],
                                ot[:])
                        else:
                            nc.vector.tensor_tensor(ot[:], reg, res_ts[md][:],
                                                    ALU.add)
                            nc.sync.dma_start(
                                out_x[md * 128:(md + 1) * 128, so:so + TT],
                                ot[:])

    nc.compile()
    return nc


# ---------------- host side ----------------

def _prep_shared(inputs, D=D, EFF=EFF):
    DC = D // 128
    EFC = EFF // 128
    f32 = np.float32
    g1 = 1.0 + np.asarray(inputs['conv_norm_g'], f32)
    g2 = 1.0 + np.asarray(inputs['gru_norm_g'], f32)
    g3 = 1.0 + np.asarray(inputs['ff_norm_g'], f32)
    conv_dw_w = np.asarray(inputs['conv_dw_w'], f32)      # [D,1,K]
    conv_dw_b = np.asarray(inputs['conv_dw_b'], f32)
    conv_pw_w = np.asarray(inputs['conv_pw_w'], f32)      # [e,d]
    conv_pw_b = np.asarray(inputs['conv_pw_b'], f32)
    gru_w = np.asarray(inputs['gru_w'], f32)              # [2D,D]
    ff_w1 = np.asarray(inputs['ff_w1'], f32)              # [4D,D]
    ff_b1 = np.asarray(inputs['ff_b1'], f32)
    ff_w2 = np.asarray(inputs['ff_w2'], f32)              # [D,4D]
    ff_b2 = np.asarray(inputs['ff_b2'], f32)

    w_dw_eff = conv_dw_w[:, 0, :] * g1[:, None]           # [D,K]
    shared = {
        'w_pw_t': np.ascontiguousarray(conv_pw_w.T),
        'w_gru_t': np.ascontiguousarray((gru_w * g2[None, :]).T),
        'w_ff1_t': np.ascontiguousarray((ff_w1 * g3[None, :]).T),
        'w_ff2_t': np.ascontiguousarray(ff_w2.T),
        'w_dw': np.ascontiguousarray(
            w_dw_eff.reshape(DC, 128, K).transpose(1, 0, 2).reshape(128, DC * K)),
        'b_eff': np.ascontiguousarray(
            (conv_pw_b + conv_pw_w @ conv_dw_b).reshape(DC, 128).T),
        'b1_in': np.ascontiguousarray(ff_b1.reshape(EFC, 128).T),
        'b2_in': np.ascontiguousarray(ff_b2.reshape(DC, 128).T),
        'ones_in': np.ones((128, 1), f32),
        'half_in': np.full((128, 1), 0.5, f32),
    }
    return shared


def _prep_device(x, b, half, T_OWN=T_OWN, BURN=BURN):
    # returns x_dev [D, T_OWN+BURN+2] and mask for device (b, half)
    f32 = np.float32
    Dl = x.shape[2]
    T_X = T_OWN + BURN + 2
    start = half * T_OWN - BURN - 2
    xd = np.zeros((Dl, T_X), f32)
    lo = max(start, 0)
    hi = half * T_OWN + T_OWN
    xd[:, lo - start:] = np.asarray(x[b, lo:hi, :], f32).T
    mask = np.full((128, 1), 0.0 if half == 0 else 1.0, f32)
    return xd, mask


_PROGRAM = None


def kernel(**inputs):
    global _PROGRAM
    from concourse.bass_utils import run_bass_kernel_spmd

    if _PROGRAM is None:
        _PROGRAM = build_program()
    nc = _PROGRAM

    shared = _prep_shared(inputs)
    x = np.asarray(inputs['x'], np.float32)

    in_maps = []
    devs = [(b, half) for b in range(B) for half in range(2)]
    for b, half in devs:
        xd, mask = _prep_device(x, b, half)
        m = dict(shared)
        m['x_in'] = xd
        m['mask_in'] = mask
        in_maps.append(m)

    res = run_bass_kernel_spmd(nc, in_maps, list(range(N_CORES))).results

    out = np.empty((B, L, D), np.float32)
    next_h = np.empty((B, 1, D), np.float32)
    for idx, (b, half) in enumerate(devs):
        out[b, half * T_OWN:(half + 1) * T_OWN, :] = res[idx]['out_x'].T
        if half == 1:
            next_h[b, 0, :] = res[idx]['out_h'][:, 0]
    return out, next_h


# revision 42
# speedup vs baseline: 49.8301x; 49.8301x over previous
"""MinGRU block kernel for Trainium2 (8 NeuronCores, data-parallel).

Sharding: 8 devices = (batch 4) x (sequence halves 2). Each device processes
2048 "own" tokens plus a 256-token burn-in prefix for the GRU scan (the linear
recurrence h_t = c*h + v with c<1 forgets its initial condition below fp32
resolution well within 256 steps), so there is no cross-device communication.
Device layout is [D, T] (channels on partitions); matmuls run in float32r.
"""
import sys
sys.path.insert(0, '/opt/trn_rl_repo')

import numpy as np
from contextlib import ExitStack

import concourse.bacc as bacc
import concourse.tile as tile
import concourse.mybir as mybir

AF = mybir.ActivationFunctionType
ALU = mybir.AluOpType
F32 = mybir.dt.float32
F32R = mybir.dt.float32r

D = 1024
EFF = 4096
T_OWN = 2048
BURN = 256
TT = 256          # token tile
K = 3             # dwconv taps
B = 4
L = 4096
N_CORES = 8
RSQRT_EPS = 1e-30


def build_program(D=D, EFF=EFF, T_OWN=T_OWN, BURN=BURN, TT=TT, TTB=384, reps=1):
    DC = D // 128            # d chunks
    EFC = EFF // 128         # ff hidden chunks
    EH = EFC // 2            # ff chunks per E-half
    T_SCAN = BURN + T_OWN
    T_X = T_SCAN + 2
    NT = T_SCAN // TTB       # A tile count
    NTB = T_SCAN // TT       # B tile count
    NTO = T_OWN // TT        # C tile count
    GRP = 2                  # Ln/Exp batching for phase A rows
    assert T_SCAN % TTB == 0 and T_SCAN % TT == 0 and T_OWN % TT == 0

    nc = bacc.Bacc("TRN2", target_bir_lowering=False, debug=False)

    # ---- DRAM I/O ----
    x_in = nc.dram_tensor("x_in", [D, T_X], F32, kind="ExternalInput").ap()
    w_pw_t = nc.dram_tensor("w_pw_t", [D, D], F32R, kind="ExternalInput").ap()
    w_gru_t = nc.dram_tensor("w_gru_t", [D, 2 * D], F32R, kind="ExternalInput").ap()
    w_ff1_t = nc.dram_tensor("w_ff1_t", [D, EFF], F32R, kind="ExternalInput").ap()
    w_ff2_t = nc.dram_tensor("w_ff2_t", [EFF, D], F32R, kind="ExternalInput").ap()
    w_dw = nc.dram_tensor("w_dw", [128, DC * K], F32, kind="ExternalInput").ap()
    b_eff = nc.dram_tensor("b_eff", [128, DC], F32, kind="ExternalInput").ap()
    b1_in = nc.dram_tensor("b1_in", [128, EFC], F32, kind="ExternalInput").ap()
    b2_in = nc.dram_tensor("b2_in", [128, DC], F32, kind="ExternalInput").ap()
    mask_in = nc.dram_tensor("mask_in", [128, 1], F32, kind="ExternalInput").ap()
    ones_in = nc.dram_tensor("ones_in", [128, 1], F32R, kind="ExternalInput").ap()
    half_in = nc.dram_tensor("half_in", [128, 1], F32, kind="ExternalInput").ap()

    out_x = nc.dram_tensor("out_x", [D, T_OWN], F32, kind="ExternalOutput").ap()
    out_h = nc.dram_tensor("out_h", [D, 1], F32, kind="ExternalOutput").ap()

    # ---- DRAM scratch ----
    x1_d = nc.dram_tensor("x1_d", [D, T_SCAN], F32).ap()
    x2_d = nc.dram_tensor("x2_d", [D, T_OWN], F32).ap()
    r2_d = nc.dram_tensor("r2_d", [1, T_SCAN], F32).ap()
    r3_d = nc.dram_tensor("r3_d", [1, T_OWN], F32).ap()
    xn32_d = nc.dram_tensor("xn32_d", [D, T_OWN], F32R).ap()
    acc_d = nc.dram_tensor("acc_d", [D, T_OWN], F32).ap()

    with tile.TileContext(nc) as tc, ExitStack() as top:
        const_pool = top.enter_context(tc.tile_pool(name="consts", bufs=1))
        ones_r = const_pool.tile([128, 1], F32R)
        nc.sync.dma_start(ones_r[:], ones_in[:])
        mask_sb = const_pool.tile([128, 1], F32)
        nc.sync.dma_start(mask_sb[:], mask_in[:])
        half_sb = const_pool.tile([128, 1], F32)
        nc.sync.dma_start(half_sb[:], half_in[:])

        for _rep in range(reps):
            # ======== Phase A: x1 = pwconv(dwconv(rmsnorm(x))) + x
            # (also emits the rmsnorm rows r1 for x and r2 for x1)
            with ExitStack() as ctx:
                wpool = ctx.enter_context(tc.tile_pool(name="A_w", bufs=1))
                xpool = ctx.enter_context(tc.tile_pool(name="A_x", bufs=2))
                rows = ctx.enter_context(tc.tile_pool(name="A_rows", bufs=1))
                work = ctx.enter_context(tc.tile_pool(name="A_work", bufs=2))
                work1 = ctx.enter_context(tc.tile_pool(name="A_work1", bufs=1))
                ps_A = ctx.enter_context(tc.tile_pool(name="A_ps", bufs=6,
                                                      space="PSUM"))

                wpw = []
                for c in range(DC):
                    wt = wpool.tile([128, D], F32R, tag=f"wpw{c}")
                    nc.sync.dma_start(wt[:], w_pw_t[c * 128:(c + 1) * 128, :])
                    wpw.append(wt)
                wdw_t = wpool.tile([128, DC * K], F32)
                nc.sync.dma_start(wdw_t[:], w_dw[:])
                beff_t = wpool.tile([128, DC], F32)
                nc.sync.dma_start(beff_t[:], b_eff[:])

                W = TTB + 2  # window including conv halo

                # --- A-pre: r1 row (rmsnorm of x), Ln/Exp batched ---
                r1_all = rows.tile([1, T_X], F32)
                for i in range(NT):
                    xo = TTB * i
                    Wi = W if i == NT - 1 else TTB
                    s_ps = ps_A.tile([1, W], F32, tag="pw")
                    for c in range(DC):
                        xp = xpool.tile([128, W], F32, tag=f"xp{c}")
                        nc.sync.dma_start(xp[:, 0:Wi],
                                          x_in[c * 128:(c + 1) * 128,
                                               xo:xo + Wi])
                        sq = work1.tile([128, W], F32R, tag=f"sq{c}")
                        nc.scalar.activation(sq[:, 0:Wi], xp[:, 0:Wi],
                                             AF.Square)
                        nc.tensor.matmul(s_ps[:, 0:Wi], ones_r[:], sq[:, 0:Wi],
                                         start=(c == 0), stop=(c == DC - 1))
                    nc.vector.tensor_scalar_add(r1_all[:, xo:xo + Wi],
                                                s_ps[:, 0:Wi], RSQRT_EPS * D)
                    if i % GRP == GRP - 1 or i == NT - 1:
                        g0 = (i // GRP) * GRP * TTB
                        g1 = xo + Wi
                        nc.scalar.activation(r1_all[:, g0:g1], r1_all[:, g0:g1],
                                             AF.Ln, scale=1.0 / D)
                        nc.scalar.activation(r1_all[:, g0:g1], r1_all[:, g0:g1],
                                             AF.Exp, scale=-0.5)

                # --- A-main ---
                s2_all = rows.tile([1, T_SCAN], F32)
                for i in range(NT):
                    xo = TTB * i
                    so = TTB * i
                    rb = work.tile([128, W], F32, tag="rb")
                    nc.gpsimd.partition_broadcast(rb[:], r1_all[:, xo:xo + W])

                    xms = []
                    xn1 = []
                    for c in range(DC):
                        xm = xpool.tile([128, W], F32, tag=f"xm{c}")
                        nc.sync.dma_start(xm[:],
                                          x_in[c * 128:(c + 1) * 128,
                                               xo:xo + W])
                        xms.append(xm)
                        xn = work.tile([128, W], F32R, tag=f"xn1_{c}")
                        nc.vector.tensor_tensor(xn[:], xm[:], rb[:], ALU.mult)
                        xn1.append(xn)

                    ys = []
                    for c in range(DC):
                        y0 = work.tile([128, TTB], F32, tag="y0")
                        nc.scalar.activation(y0[:], xn1[c][:, 0:TTB].bitcast(F32),
                                             AF.Copy,
                                             scale=wdw_t[:, c * K:c * K + 1])
                        y1 = work.tile([128, TTB], F32, tag="y1")
                        nc.vector.scalar_tensor_tensor(
                            y1[:], xn1[c][:, 1:TTB + 1].bitcast(F32),
                            wdw_t[:, c * K + 1:c * K + 2], y0[:],
                            ALU.mult, ALU.add)
                        y2 = work.tile([128, TTB], F32R, tag=f"y2_{c}")
                        nc.vector.scalar_tensor_tensor(
                            y2[:], xn1[c][:, 2:TTB + 2].bitcast(F32),
                            wdw_t[:, c * K + 2:c * K + 3], y1[:],
                            ALU.mult, ALU.add)
                        ys.append(y2)

                    s2_ps = ps_A.tile([1, TTB], F32, tag="pw")
                    for m in range(DC):
                        pw_ps = ps_A.tile([128, TTB], F32, tag="pw")
                        for c in range(DC):
                            nc.tensor.matmul(pw_ps[:],
                                             wpw[c][:, m * 128:(m + 1) * 128],
                                             ys[c][:], start=(c == 0),
                                             stop=(c == DC - 1))
                        x1t = work.tile([128, TTB], F32, tag=f"x1t{m}")
                        nc.vector.scalar_tensor_tensor(
                            x1t[:], pw_ps[:], beff_t[:, m:m + 1],
                            xms[m][:, 2:2 + TTB], ALU.add, ALU.add)
                        nc.sync.dma_start(
                            x1_d[m * 128:(m + 1) * 128, so:so + TTB], x1t[:])
                        sq2 = work.tile([128, TTB], F32R, tag="sq2")
                        nc.scalar.activation(sq2[:], x1t[:], AF.Square)
                        nc.tensor.matmul(s2_ps[:], ones_r[:], sq2[:],
                                         start=(m == 0), stop=(m == DC - 1))
                    nc.vector.tensor_scalar_add(s2_all[:, so:so + TTB],
                                                s2_ps[:], RSQRT_EPS * D)
                    if i % GRP == GRP - 1 or i == NT - 1:
                        g0 = (i // GRP) * GRP * TTB
                        g1 = so + TTB
                        nc.scalar.activation(s2_all[:, g0:g1], s2_all[:, g0:g1],
                                             AF.Ln, scale=1.0 / D)
                        nc.scalar.activation(s2_all[:, g0:g1], s2_all[:, g0:g1],
                                             AF.Exp, scale=-0.5)
                        nc.sync.dma_start(r2_d[:, g0:g1], s2_all[:, g0:g1])

            # ======== Phase B: GRU + fused rmsnorm(x2) -> x2_d, xn3_d, out_h
            with ExitStack() as ctx:
                wpool = ctx.enter_context(tc.tile_pool(name="B_w", bufs=1))
                pa = ctx.enter_context(tc.tile_pool(name="B_a", bufs=2))
                pb = ctx.enter_context(tc.tile_pool(name="B_b", bufs=2))
                pc1 = ctx.enter_context(tc.tile_pool(name="B_c", bufs=1))
                pd = ctx.enter_context(tc.tile_pool(name="B_d", bufs=2))
                ps_B = ctx.enter_context(tc.tile_pool(name="B_ps", bufs=5,
                                                      space="PSUM"))

                wgru = []
                for c in range(DC):
                    wt = wpool.tile([128, 2 * D], F32R, tag=f"wgru{c}")
                    nc.sync.dma_start(wt[:], w_gru_t[c * 128:(c + 1) * 128, :])
                    wgru.append(wt)

                h_prev = [None] * DC
                for i in range(NTB):
                    so = TT * i
                    x1ts = []
                    for c in range(DC):
                        x1t = pd.tile([128, TT], F32, tag=f"x1t{c}")
                        nc.sync.dma_start(
                            x1t[:], x1_d[c * 128:(c + 1) * 128, so:so + TT])
                        x1ts.append(x1t)
                    r2row = pa.tile([1, TT], F32, tag="r2row")
                    nc.sync.dma_start(r2row[:], r2_d[:, so:so + TT])
                    rb2 = pa.tile([128, TT], F32, tag="rb2")
                    nc.gpsimd.partition_broadcast(rb2[:], r2row[:])
                    xn2 = []
                    for c in range(DC):
                        xn = pa.tile([128, TT], F32R, tag=f"xn2_{c}")
                        nc.vector.tensor_tensor(xn[:], x1ts[c][:], rb2[:],
                                                ALU.mult)
                        xn2.append(xn)

                    gs = []
                    for m in range(DC):           # hidden chunks
                        hg_ps = ps_B.tile([128, TT], F32, tag="hg")
                        for c in range(DC):
                            nc.tensor.matmul(hg_ps[:],
                                             wgru[c][:, m * 128:(m + 1) * 128],
                                             xn2[c][:], start=(c == 0),
                                             stop=(c == DC - 1))
                        sg = pa.tile([128, TT], F32, tag="sg")
                        nc.scalar.activation(sg[:], hg_ps[:], AF.Sigmoid)
                        # g = max(hidden + 0.5, sigmoid(hidden))  (exact)
                        g = pc1.tile([128, TT], F32, tag=f"g{m}")
                        nc.vector.scalar_tensor_tensor(
                            g[:], hg_ps[:], half_sb[:], sg[:], ALU.add, ALU.max)
                        gs.append(g)

                    hs = []
                    for mc in range(DC):          # gate chunks
                        m = DC + mc
                        hg_ps = ps_B.tile([128, TT], F32, tag="hg")
                        for c in range(DC):
                            nc.tensor.matmul(hg_ps[:],
                                             wgru[c][:, m * 128:(m + 1) * 128],
                                             xn2[c][:], start=(c == 0),
                                             stop=(c == DC - 1))
                        ct = pb.tile([128, TT], F32, tag=f"ct{mc}")
                        nc.scalar.activation(ct[:], hg_ps[:], AF.Sigmoid,
                                             scale=-1.0)
                        z = pa.tile([128, TT], F32, tag="z")
                        nc.scalar.activation(z[:], hg_ps[:], AF.Sigmoid)
                        v = pc1.tile([128, TT], F32, tag=f"v{mc}")
                        nc.vector.tensor_tensor(v[:], z[:], gs[mc][:], ALU.mult)
                        if i == BURN // TT:
                            # decay reset at the first real token (mask=0 on
                            # half-0 devices; no-op when mask=1)
                            mo = BURN % TT
                            nc.vector.tensor_scalar_mul(ct[:, mo:mo + 1],
                                                        ct[:, mo:mo + 1],
                                                        mask_sb[:])
                        h = pd.tile([128, TT], F32, tag=f"h{mc}")
                        init = 0.0 if i == 0 else h_prev[mc][:, TT - 1:TT]
                        nc.vector.tensor_tensor_scan(h[:], ct[:], v[:], init,
                                                     ALU.mult, ALU.add)
                        hs.append(h)
                        lo = max(BURN - so, 0)
                        if lo < TT:
                            x2t = pb.tile([128, TT], F32, tag=f"x2t{mc}")
                            nc.vector.tensor_tensor(x2t[:, lo:TT],
                                                    h[:, lo:TT],
                                                    x1ts[mc][:, lo:TT],
                                                    ALU.add)
                            nc.sync.dma_start(
                                x2_d[mc * 128:(mc + 1) * 128,
                                     so + lo - BURN:so - BURN + TT],
                                x2t[:, lo:TT])
                        if i == NTB - 1:
                            nc.sync.dma_start(out_h[mc * 128:(mc + 1) * 128, :],
                                              hs[mc][:, TT - 1:TT])
                    h_prev = hs

            # ======== Phase C: out = W2 gelu(W1 xn3 + b1) + b2 + x2
            # E split in 2 halves; half-0 partial (+b2+x2) -> acc_d; half-1
            # adds it back and writes out_x.
            with ExitStack() as cctx:
                wpool = cctx.enter_context(tc.tile_pool(name="C_w", bufs=1))
                b1t = wpool.tile([128, EFC], F32)
                nc.sync.dma_start(b1t[:], b1_in[:])
                b2t = wpool.tile([128, DC], F32)
                nc.sync.dma_start(b2t[:], b2_in[:])

                def load_w(eh):
                    w1s, w2s = [], []
                    for c in range(DC):
                        w1t = wpool.tile([128, EFF // 2], F32R, tag=f"w1_{c}")
                        nc.sync.dma_start(
                            w1t[:],
                            w_ff1_t[c * 128:(c + 1) * 128,
                                    eh * (EFF // 2):(eh + 1) * (EFF // 2)])
                        w1s.append(w1t)
                    for e in range(EH):
                        ge = eh * EH + e
                        w2t = wpool.tile([128, D], F32R, tag=f"w2_{e}")
                        nc.sync.dma_start(w2t[:],
                                          w_ff2_t[ge * 128:(ge + 1) * 128, :])
                        w2s.append(w2t)
                    return w1s, w2s

                work2 = cctx.enter_context(tc.tile_pool(name="C_work2", bufs=2))
                h1p = cctx.enter_context(tc.tile_pool(name="C_h1", bufs=8))


                weights = [load_w(0)]

                rctx = ExitStack()
                postp = rctx.enter_context(tc.tile_pool(name="C_post", bufs=2))
                rows = rctx.enter_context(tc.tile_pool(name="C_rows", bufs=1))
                ps_s3 = rctx.enter_context(
                    tc.tile_pool(name="C_ps_s3", bufs=2, space="PSUM"))
                r3_all = rows.tile([1, T_OWN], F32)
                # --- rmsnorm(x2) rows (continues phase B's output) ---
                GRP3 = 4
                for io in range(NTO):
                    oo = TT * io
                    x2ts = []
                    for c in range(DC):
                        x2t = postp.tile([128, TT], F32, tag=f"x2r{c}")
                        nc.sync.dma_start(
                            x2t[:], x2_d[c * 128:(c + 1) * 128, oo:oo + TT])
                        x2ts.append(x2t)
                    s3_ps = ps_s3.tile([1, TT], F32, tag="s3")
                    for mc in range(DC):
                        sq3 = rows.tile([128, TT], F32R, tag="sq3")
                        nc.scalar.activation(sq3[:], x2ts[mc][:], AF.Square)
                        nc.tensor.matmul(s3_ps[:], ones_r[:], sq3[:],
                                         start=(mc == 0), stop=(mc == DC - 1))
                    nc.vector.tensor_scalar_add(r3_all[:, oo:oo + TT],
                                                s3_ps[:], RSQRT_EPS * D)
                    if io % GRP3 == GRP3 - 1 or io == NTO - 1:
                        b0 = (io // GRP3) * GRP3
                        nc.scalar.activation(r3_all[:, b0 * TT:oo + TT],
                                             r3_all[:, b0 * TT:oo + TT],
                                             AF.Ln, scale=1.0 / D)
                        nc.scalar.activation(r3_all[:, b0 * TT:oo + TT],
                                             r3_all[:, b0 * TT:oo + TT],
                                             AF.Exp, scale=-0.5)
                        nc.sync.dma_start(r3_d[:, b0 * TT:oo + TT],
                                          r3_all[:, b0 * TT:oo + TT])


                rctx.close()
                ps_f1 = cctx.enter_context(
                    tc.tile_pool(name="C_ps1", bufs=4, space="PSUM"))
                ps_f2 = cctx.enter_context(
                    tc.tile_pool(name="C_ps2", bufs=1, space="PSUM"))
                for eh in range(2):
                    if eh > 0:
                        weights.append(load_w(eh))
                    w1s, w2s = weights[eh]
                    for i in range(NTO):
                        so = TT * i
                        res_ts = []
                        for c in range(DC):
                            rt = work2.tile([128, TT], F32, tag=f"res{c}")
                            src_d = x2_d if eh == 0 else acc_d
                            nc.sync.dma_start(
                                rt[:], src_d[c * 128:(c + 1) * 128, so:so + TT])
                            res_ts.append(rt)
                        xn3ts = []
                        if eh == 0:
                            r3row = work2.tile([1, TT], F32, tag="r3row")
                            nc.sync.dma_start(r3row[:], r3_d[:, so:so + TT])
                            rb3 = work2.tile([128, TT], F32, tag="rb3c")
                            nc.gpsimd.partition_broadcast(rb3[:], r3row[:])
                            for c in range(DC):
                                xt = work2.tile([128, TT], F32R, tag=f"xn3t{c}")
                                nc.vector.tensor_tensor(xt[:], res_ts[c][:],
                                                        rb3[:], ALU.mult)
                                nc.sync.dma_start(
                                    xn32_d[c * 128:(c + 1) * 128, so:so + TT],
                                    xt[:])
                                xn3ts.append(xt)
                        else:
                            for c in range(DC):
                                xt = work2.tile([128, TT], F32R, tag=f"xn3t{c}")
                                nc.sync.dma_start(
                                    xt[:],
                                    xn32_d[c * 128:(c + 1) * 128, so:so + TT])
                                xn3ts.append(xt)

                        f2_ps = []
                        for j in range(DC // 2):
                            f2b = ps_f2.tile([128, 2 * TT], F32, tag=f"f2_{j}")
                            f2_ps.append(f2b)
                        for me in range(EH):
                            f1 = ps_f1.tile([128, TT], F32, tag="f1")
                            for c in range(DC):
                                nc.tensor.matmul(
                                    f1[:], w1s[c][:, me * 128:(me + 1) * 128],
                                    xn3ts[c][:], start=(c == 0),
                                    stop=(c == DC - 1))
                            h1 = h1p.tile([128, TT], F32R, tag="h1")
                            nc.scalar.activation(
                                h1[:], f1[:], AF.Gelu,
                                bias=b1t[:, eh * EH + me:eh * EH + me + 1])
                            for md in range(DC):
                                reg = f2_ps[md // 2][:, (md % 2) * TT:
                                                     (md % 2 + 1) * TT]
                                nc.tensor.matmul(
                                    reg, w2s[me][:, md * 128:(md + 1) * 128],
                                    h1[:],
                                    start=(me == 0 and md % 2 == 0),
                                    stop=(me == EH - 1 and md % 2 == 1))
                        for md in range(DC):
                            reg = f2_ps[md // 2][:, (md % 2) * TT:
                                                 (md % 2 + 1) * TT]
                            ot = work2.tile([128, TT], F32, tag="ot")
                            if eh == 0:
                                nc.vector.scalar_tensor_tensor(
                                    ot[:], reg, b2t[:, md:md + 1],
                                    res_ts[md][:], ALU.add, ALU.add)
                                nc.sync.dma_start(
                                    acc_d[md * 128:(md + 1) * 128, so:so + TT],
                                    ot[:])
                            else:
                                nc.vector.tensor_tensor(ot[:], reg,
                                                        res_ts[md][:], ALU.add)
                                nc.sync.dma_start(
                                    out_x[md * 128:(md + 1) * 128, so:so + TT],
                                    ot[:])

    nc.compile()
    return nc


# ---------------- host side ----------------

def _prep_shared(inputs, D=D, EFF=EFF):
    DC = D // 128
    EFC = EFF // 128
    f32 = np.float32
    g1 = 1.0 + np.asarray(inputs['conv_norm_g'], f32)
    g2 = 1.0 + np.asarray(inputs['gru_norm_g'], f32)
    g3 = 1.0 + np.asarray(inputs['ff_norm_g'], f32)
    conv_dw_w = np.asarray(inputs['conv_dw_w'], f32)      # [D,1,K]
    conv_dw_b = np.asarray(inputs['conv_dw_b'], f32)
    conv_pw_w = np.asarray(inputs['conv_pw_w'], f32)      # [e,d]
    conv_pw_b = np.asarray(inputs['conv_pw_b'], f32)
    gru_w = np.asarray(inputs['gru_w'], f32)              # [2D,D]
    ff_w1 = np.asarray(inputs['ff_w1'], f32)              # [4D,D]
    ff_b1 = np.asarray(inputs['ff_b1'], f32)
    ff_w2 = np.asarray(inputs['ff_w2'], f32)              # [D,4D]
    ff_b2 = np.asarray(inputs['ff_b2'], f32)

    w_dw_eff = conv_dw_w[:, 0, :] * g1[:, None]           # [D,K]
    shared = {
        'w_pw_t': np.ascontiguousarray(conv_pw_w.T),
        'w_gru_t': np.ascontiguousarray((gru_w * g2[None, :]).T),
        'w_ff1_t': np.ascontiguousarray((ff_w1 * g3[None, :]).T),
        'w_ff2_t': np.ascontiguousarray(ff_w2.T),
        'w_dw': np.ascontiguousarray(
            w_dw_eff.reshape(DC, 128, K).transpose(1, 0, 2).reshape(128, DC * K)),
        'b_eff': np.ascontiguousarray(
            (conv_pw_b + conv_pw_w @ conv_dw_b).reshape(DC, 128).T),
        'b1_in': np.ascontiguousarray(ff_b1.reshape(EFC, 128).T),
        'b2_in': np.ascontiguousarray(ff_b2.reshape(DC, 128).T),
        'ones_in': np.ones((128, 1), f32),
        'half_in': np.full((128, 1), 0.5, f32),
    }
    return shared


def _prep_device(x, b, half, T_OWN=T_OWN, BURN=BURN):
    f32 = np.float32
    Dl = x.shape[2]
    T_X = T_OWN + BURN + 2
    start = half * T_OWN - BURN - 2
    xd = np.zeros((Dl, T_X), f32)
    lo = max(start, 0)
    hi = half * T_OWN + T_OWN
    xd[:, lo - start:] = np.asarray(x[b, lo:hi, :], f32).T
    mask = np.full((128, 1), 0.0 if half == 0 else 1.0, f32)
    return xd, mask


_PROGRAM = None


def kernel(**inputs):
    global _PROGRAM
    from concourse.bass_utils import run_bass_kernel_spmd

    if _PROGRAM is None:
        _PROGRAM = build_program()
    nc = _PROGRAM

    shared = _prep_shared(inputs)
    x = np.asarray(inputs['x'], np.float32)

    in_maps = []
    devs = [(b, half) for b in range(B) for half in range(2)]
    for b, half in devs:
        xd, mask = _prep_device(x, b, half)
        m = dict(shared)
        m['x_in'] = xd
        m['mask_in'] = mask
        in_maps.append(m)

    res = run_bass_kernel_spmd(nc, in_maps, list(range(N_CORES))).results

    out = np.empty((B, L, D), np.float32)
    next_h = np.empty((B, 1, D), np.float32)
    for idx, (b, half) in enumerate(devs):
        out[b, half * T_OWN:(half + 1) * T_OWN, :] = res[idx]['out_x'].T
        if half == 1:
            next_h[b, 0, :] = res[idx]['out_h'][:, 0]
    return out, next_h
